# revision 3
# baseline (speedup 1.0000x reference)
"""CenterLoss kernel for Trainium2 (8 NeuronCores, SPMD).

Reference computes
    distmat[b,c] = ||x_b||^2 + ||c_c||^2 - 2<x_b, c_c>          [B, C]
    loss = sum(clip(distmat * onehot(labels), 1e-12, 1e12)) / B

Only distmat[b, labels[b]] survives the mask, so the loss needs
d_b = ||x_b - centers[l_b]||^2 per row plus the closed-form constant
B*(C-1)*1e-12 for the masked zeros the clip turns into 1e-12.

Architecture: class-sharded centers + aligned x placement + sum-collapse
(~21.6-22.3us vs 28.0-28.6us for the gather-everything baseline, which is
hard-floored at ~11us of Q7 SWDGE descriptor emission: 1024 rows/core x
~8.5ns/descriptor, 128 rows max per indirect call).

  - core k owns classes [1250k, 1250(k+1)) (sharding_hint's column
    sharding); batch rows are routed to their label's owner core (pure
    host-side index work). The FIRST row of each distinct class sits at
    the class's slot in [c|x]-interleaved buffers; duplicate-class rows
    (seed-0 max 351/core) go to a 384-slot overflow served by 3 indirect
    gathers.
  - sum-collapse: the host only ever SUMS distances (clip(d,1e-12) is a
    no-op for real rows, d ~ chi^2(256)*2 ~ 512), so empty aligned slots
    are padded x := c (bf16-identical -> exact 0) and empty overflow
    slots x := centers[0] (what jo=0 gathers). Every pad contributes 0;
    each region piece reduces on-device via one ACT Square+accum_out and
    overflow via DVE sub+mul+reduce. Output: [128, 6] f32 per core.
  - region split (5,3,2) slots: three SP-ring FIFO DMAs arrive ~2us
    apart, so ACT square work pipelines behind the stream instead of
    stalling on one late load.
  - add_dep_helper ordering (overflow-j after region-sub) stops the Tile
    scheduler from hoisting gather-fed work ahead of ready region work on
    the in-order DVE (its cost model underestimates gather completion
    latency by ~2.5us).
  - rows beyond region+overflow capacity spill to additional identical
    invocations (correct for any input; never triggers for uniform labels).

Hard-won HW constraints baked in (this runtime rejects/crashes otherwise):
  - Bacc + finalize() before run; stock instructions only (custom "Ant"
    ISA ops kill the exec unit); no in-place DVE ops.
  - indirect_dma_start: offset AP must be a [128, 1] column, dest a whole
    [128, D] tile; multi-column offsets / partition-sliced dests gather
    garbage (verified: ucode reads wrong index positions).
  - Q7 emission is per-descriptor (~8.5ns) regardless of element size;
    SWDGE gather data drains only after the concurrent HWDGE backlog and
    its semaphore lags the last packet by ~1.5-2.5us.
"""

import numpy as np

from concourse import bacc, bass, mybir
import concourse.tile as tile
from concourse.bass_utils import run_bass_kernel_spmd

B = 8192
C = 10000
D = 256
N_CORES = 8
P = 128

CPC = C // N_CORES          # classes per core (1250)
S = (CPC + P - 1) // P      # region slots per partition (10)
CPAD = P * S                # padded classes per core (1280)
OVF = 384                   # overflow rows per core (seed-0 max is 351)
J = OVF // P                # overflow gather calls (3)
H = S // 2                  # region half (5 slots)

_CLIP_LO = 1e-12

_nc_cache = None


def _build():
    global _nc_cache
    if _nc_cache is not None:
        return _nc_cache

    nc = bacc.Bacc()
    # cx*: [c|x] interleaved per (partition, slot) -> 5KB contiguous
    # per-partition runs, one completion receipt per half.
    cxa = nc.dram_tensor("cxa", [P, 5, 2, D], mybir.dt.bfloat16, kind="ExternalInput")
    cxb = nc.dram_tensor("cxb", [P, 3, 2, D], mybir.dt.bfloat16, kind="ExternalInput")
    cxc = nc.dram_tensor("cxc", [P, 2, 2, D], mybir.dt.bfloat16, kind="ExternalInput")
    xo = nc.dram_tensor("xovf", [OVF, D], mybir.dt.bfloat16, kind="ExternalInput")
    jo = nc.dram_tensor("jovf", [OVF], mybir.dt.int32, kind="ExternalInput")
    cen = nc.dram_tensor("centers", [C, D], mybir.dt.bfloat16, kind="ExternalInput")
    out = nc.dram_tensor("sums", [P, 3 + J], mybir.dt.float32, kind="ExternalOutput")

    with tile.TileContext(nc) as tc:
        with (
            tc.tile_pool(name="big", bufs=1) as big,
            tc.tile_pool(name="work", bufs=4) as work,
            tc.tile_pool(name="gtp", bufs=J) as gtp,
        ):
            jt = big.tile([P, J], mybir.dt.int32)
            cxat = big.tile([P, 5, 2, D], mybir.dt.bfloat16)
            cxbt = big.tile([P, 3, 2, D], mybir.dt.bfloat16)
            cxct = big.tile([P, 2, 2, D], mybir.dt.bfloat16)
            xot = big.tile([P, J, D], mybir.dt.bfloat16)
            acc = big.tile([P, 3 + J], mybir.dt.float32)

            # SP ring FIFO: jt -> cxa -> cxb; xot parallel on the ACT ring
            nc.sync.dma_start(out=jt[:], in_=jo[:].rearrange("(p j) -> p j", j=J))
            nc.sync.dma_start(out=cxat[:], in_=cxa[:])
            nc.sync.dma_start(out=cxbt[:], in_=cxb[:])
            nc.sync.dma_start(out=cxct[:], in_=cxc[:])
            nc.scalar.dma_start(
                out=xot[:], in_=xo[:].rearrange("(p j) d -> p j d", j=J)
            )

            gts = []
            for j in range(J):
                gt = gtp.tile([P, D], mybir.dt.bfloat16, tag="gt")
                nc.gpsimd.indirect_dma_start(
                    out=gt[:],
                    out_offset=None,
                    in_=cen[:],
                    in_offset=bass.IndirectOffsetOnAxis(ap=jt[:, j:j + 1], axis=0),
                )
                gts.append(gt)

            def region(cxt, w, col, tag):
                dtc = work.tile([P, w, D], mybir.dt.bfloat16, tag=f"dt{tag}")
                sub_inst = nc.vector.tensor_sub(
                    out=dtc[:], in0=cxt[:, :, 1, :], in1=cxt[:, :, 0, :]
                )
                sqc = work.tile([P, w, D], mybir.dt.bfloat16, tag=f"sq{tag}")
                nc.scalar.activation(
                    out=sqc[:],
                    in_=dtc[:],
                    func=mybir.ActivationFunctionType.Square,
                    accum_out=acc[:, col:col + 1],
                )
                return sub_inst

            def ovf(j, after=None):
                dt = work.tile([P, D], mybir.dt.bfloat16, tag="dto")
                sub_inst = nc.vector.tensor_sub(
                    out=dt[:], in0=xot[:, j, :], in1=gts[j][:]
                )
                if after is not None:
                    tile.add_dep_helper(
                        sub_inst.ins,
                        after.ins,
                        sync=False,
                        reason="keep region subs ahead of overflow on DVE",
                    )
                sq = work.tile([P, D], mybir.dt.bfloat16, tag="sqo")
                nc.vector.tensor_mul(out=sq[:], in0=dt[:], in1=dt[:])
                nc.vector.tensor_reduce(
                    out=acc[:, 3 + j:4 + j],
                    in_=sq[:],
                    axis=mybir.AxisListType.X,
                    op=mybir.AluOpType.add,
                )

            region(cxat, 5, 0, "a")
            sub_b1 = region(cxbt, 3, 1, "b")
            sub_b2 = region(cxct, 2, 2, "c")
            ovf(0, after=sub_b1)
            ovf(1, after=sub_b2)
            ovf(2)

            nc.sync.dma_start(out=out[:], in_=acc[:])

    nc.finalize()
    _nc_cache = nc
    return nc


def _pack_core(x_bf, cs_k, cen0, lab, rows, k):
    """Pack one core's rows. Empty region slots get x:=c (contributes 0);
    empty overflow slots get x:=centers[0], jo:=0 (contributes 0)."""
    m = lab[rows] - k * CPC
    order = np.argsort(m, kind="stable")
    rs = rows[order]
    ms = m[order]
    first = np.ones(len(ms), dtype=bool)
    first[1:] = ms[1:] != ms[:-1]

    xr = cs_k.copy()  # empty slots: x == c -> exactly 0
    xr[ms[first]] = x_bf[rs[first]]

    rest = rs[~first]
    ovf_rows = rest[:OVF]
    leftover = rest[OVF:]
    xo = np.broadcast_to(cen0, (OVF, D)).copy()
    jo = np.zeros(OVF, dtype=np.int32)
    xo[: len(ovf_rows)] = x_bf[ovf_rows]
    jo[: len(ovf_rows)] = lab[ovf_rows]
    return xr, xo, jo, leftover


def _run(x, labels, centers, **spmd_kwargs):
    import jax.numpy as jnp

    nc = _build()
    x = np.ascontiguousarray(np.asarray(x), dtype=np.float32)
    labels = np.ascontiguousarray(np.asarray(labels)).astype(np.int64)
    centers = np.ascontiguousarray(np.asarray(centers), dtype=np.float32)

    bf = jnp.bfloat16
    x_bf = np.asarray(jnp.asarray(x, dtype=bf))
    cen_bf = np.asarray(jnp.asarray(centers, dtype=bf))
    cs_pad = np.zeros((N_CORES, CPAD, D), dtype=cen_bf.dtype)
    cs_pad[:, :CPC] = cen_bf.reshape(N_CORES, CPC, D)

    owner = labels // CPC
    pending = [np.flatnonzero(owner == k) for k in range(N_CORES)]

    total = 0.0
    res = None
    while any(len(r) for r in pending):
        in_maps = []
        next_pending = []
        for k in range(N_CORES):
            xr, xo, jo, leftover = _pack_core(
                x_bf, cs_pad[k], cen_bf[0], labels, pending[k], k
            )
            cs3 = cs_pad[k].reshape(P, S, D)
            xr3 = xr.reshape(P, S, D)
            in_maps.append(
                {
                    "cxa": np.ascontiguousarray(
                        np.stack([cs3[:, 0:5], xr3[:, 0:5]], axis=2)
                    ),
                    "cxb": np.ascontiguousarray(
                        np.stack([cs3[:, 5:8], xr3[:, 5:8]], axis=2)
                    ),
                    "cxc": np.ascontiguousarray(
                        np.stack([cs3[:, 8:10], xr3[:, 8:10]], axis=2)
                    ),
                    "xovf": xo,
                    "jovf": jo,
                    "centers": cen_bf,
                }
            )
            next_pending.append(leftover)
        res = run_bass_kernel_spmd(nc, in_maps, list(range(N_CORES)), **spmd_kwargs)
        for k in range(N_CORES):
            total += res.results[k]["sums"].astype(np.float64).sum()
        pending = next_pending
        spmd_kwargs = {}

    loss = (total + B * (C - 1) * _CLIP_LO) / B
    return np.asarray(loss, dtype=np.float32), res


def kernel(x, labels, centers):
    loss, _ = _run(x, labels, centers)
    return loss


# revision 4
# speedup vs baseline: 1.0855x; 1.0855x over previous
"""CenterLoss kernel for Trainium2 (8 NeuronCores, SPMD).

Reference computes
    distmat[b,c] = ||x_b||^2 + ||c_c||^2 - 2<x_b, c_c>          [B, C]
    loss = sum(clip(distmat * onehot(labels), 1e-12, 1e12)) / B

Only distmat[b, labels[b]] survives the mask, so the loss needs
d_b = ||x_b - centers[l_b]||^2 per row plus the closed-form constant
B*(C-1)*1e-12 for the masked zeros the clip turns into 1e-12.

Architecture: class-sharded centers + aligned x placement + sum-collapse
(~21.6-22.3us vs 28.0-28.6us for the gather-everything baseline, which is
hard-floored at ~11us of Q7 SWDGE descriptor emission: 1024 rows/core x
~8.5ns/descriptor, 128 rows max per indirect call).

  - core k owns classes [1250k, 1250(k+1)) (sharding_hint's column
    sharding); batch rows are routed to their label's owner core (pure
    host-side index work). The FIRST row of each distinct class sits at
    the class's slot in [c|x]-interleaved buffers; duplicate-class rows
    (seed-0 max 351/core) go to a 384-slot overflow served by 3 indirect
    gathers.
  - sum-collapse: the host only ever SUMS distances (clip(d,1e-12) is a
    no-op for real rows, d ~ chi^2(256)*2 ~ 512), so empty aligned slots
    are padded x := c (bf16-identical -> exact 0) and empty overflow
    slots x := centers[0] (what jo=0 gathers). Every pad contributes 0;
    each region piece reduces on-device via one ACT Square+accum_out and
    overflow via DVE sub+mul+reduce. Output: [128, 6] f32 per core.
  - region split (5,3,2) slots: three SP-ring FIFO DMAs arrive ~2us
    apart, so ACT square work pipelines behind the stream instead of
    stalling on one late load.
  - add_dep_helper ordering (overflow-j after region-sub) stops the Tile
    scheduler from hoisting gather-fed work ahead of ready region work on
    the in-order DVE (its cost model underestimates gather completion
    latency by ~2.5us).
  - rows beyond region+overflow capacity spill to additional identical
    invocations (correct for any input; never triggers for uniform labels).

Hard-won HW constraints baked in (this runtime rejects/crashes otherwise):
  - Bacc + finalize() before run; stock instructions only (custom "Ant"
    ISA ops kill the exec unit); no in-place DVE ops.
  - indirect_dma_start: offset AP must be a [128, 1] column, dest a whole
    [128, D] tile; multi-column offsets / partition-sliced dests gather
    garbage (verified: ucode reads wrong index positions).
  - Q7 emission is per-descriptor (~8.5ns) regardless of element size;
    SWDGE gather data drains only after the concurrent HWDGE backlog and
    its semaphore lags the last packet by ~1.5-2.5us.
"""

import numpy as np

from concourse import bacc, bass, mybir
import concourse.tile as tile
from concourse.bass_utils import run_bass_kernel_spmd

B = 8192
C = 10000
D = 256
N_CORES = 8
P = 128

CPC = C // N_CORES          # classes per core (1250)
S = (CPC + P - 1) // P      # region slots per partition (10)
CPAD = P * S                # padded classes per core (1280)
OVF = 384                   # overflow rows per core (seed-0 max is 351)
J = OVF // P                # overflow gather calls (3)
H = S // 2                  # region half (5 slots)

_CLIP_LO = 1e-12

_nc_cache = None


def _build():
    global _nc_cache
    if _nc_cache is not None:
        return _nc_cache

    nc = bacc.Bacc()
    # cx*: [c|x] interleaved per (partition, slot) -> 5KB contiguous
    # per-partition runs, one completion receipt per half.
    cxa = nc.dram_tensor("cxa", [P, 5, 2, D], mybir.dt.bfloat16, kind="ExternalInput")
    cxb = nc.dram_tensor("cxb", [P, 3, 2, D], mybir.dt.bfloat16, kind="ExternalInput")
    cxc = nc.dram_tensor("cxc", [P, 2, 2, D], mybir.dt.bfloat16, kind="ExternalInput")
    xo = nc.dram_tensor("xovf", [OVF, D], mybir.dt.bfloat16, kind="ExternalInput")
    jo = nc.dram_tensor("jovf", [OVF], mybir.dt.int32, kind="ExternalInput")
    # compact per-core gather source: overflow labels are all local
    # classes, and 384 random 512B reads inside a 0.64MB shard hit HBM row
    # buffers far more often than inside the full 5MB table (~0.4us faster
    # gather completion)
    cen = nc.dram_tensor("cshard", [CPAD, D], mybir.dt.bfloat16, kind="ExternalInput")
    out = nc.dram_tensor("sums", [P, 3 + J], mybir.dt.float32, kind="ExternalOutput")

    with tile.TileContext(nc) as tc:
        with (
            tc.tile_pool(name="big", bufs=1) as big,
            tc.tile_pool(name="work", bufs=4) as work,
            tc.tile_pool(name="gtp", bufs=J) as gtp,
        ):
            jt = big.tile([P, J], mybir.dt.int32)
            cxat = big.tile([P, 5, 2, D], mybir.dt.bfloat16)
            cxbt = big.tile([P, 3, 2, D], mybir.dt.bfloat16)
            cxct = big.tile([P, 2, 2, D], mybir.dt.bfloat16)
            xot = big.tile([P, J, D], mybir.dt.bfloat16)
            acc = big.tile([P, 3 + J], mybir.dt.float32)

            # SP ring FIFO: jt -> cxa -> cxb; xot parallel on the ACT ring
            nc.sync.dma_start(out=jt[:], in_=jo[:].rearrange("(p j) -> p j", j=J))
            nc.sync.dma_start(out=cxat[:], in_=cxa[:])
            nc.sync.dma_start(out=cxbt[:], in_=cxb[:])
            nc.sync.dma_start(out=cxct[:], in_=cxc[:])
            nc.scalar.dma_start(
                out=xot[:], in_=xo[:].rearrange("(p j) d -> p j d", j=J)
            )

            gts = []
            for j in range(J):
                gt = gtp.tile([P, D], mybir.dt.bfloat16, tag="gt")
                nc.gpsimd.indirect_dma_start(
                    out=gt[:],
                    out_offset=None,
                    in_=cen[:],
                    in_offset=bass.IndirectOffsetOnAxis(ap=jt[:, j:j + 1], axis=0),
                )
                gts.append(gt)

            def region(cxt, w, col, tag):
                dtc = work.tile([P, w, D], mybir.dt.bfloat16, tag=f"dt{tag}")
                sub_inst = nc.vector.tensor_sub(
                    out=dtc[:], in0=cxt[:, :, 1, :], in1=cxt[:, :, 0, :]
                )
                sqc = work.tile([P, w, D], mybir.dt.bfloat16, tag=f"sq{tag}")
                nc.scalar.activation(
                    out=sqc[:],
                    in_=dtc[:],
                    func=mybir.ActivationFunctionType.Square,
                    accum_out=acc[:, col:col + 1],
                )
                return sub_inst

            def ovf(j, after=None):
                dt = work.tile([P, D], mybir.dt.bfloat16, tag="dto")
                sub_inst = nc.vector.tensor_sub(
                    out=dt[:], in0=xot[:, j, :], in1=gts[j][:]
                )
                if after is not None:
                    tile.add_dep_helper(
                        sub_inst.ins,
                        after.ins,
                        sync=False,
                        reason="keep region subs ahead of overflow on DVE",
                    )
                sq = work.tile([P, D], mybir.dt.bfloat16, tag="sqo")
                nc.vector.tensor_mul(out=sq[:], in0=dt[:], in1=dt[:])
                nc.vector.tensor_reduce(
                    out=acc[:, 3 + j:4 + j],
                    in_=sq[:],
                    axis=mybir.AxisListType.X,
                    op=mybir.AluOpType.add,
                )

            region(cxat, 5, 0, "a")
            sub_b1 = region(cxbt, 3, 1, "b")
            sub_b2 = region(cxct, 2, 2, "c")
            ovf(0, after=sub_b1)
            ovf(1, after=sub_b2)
            ovf(2)

            nc.sync.dma_start(out=out[:], in_=acc[:])

    nc.finalize()
    _nc_cache = nc
    return nc


def _pack_core(x_bf, cs_k, cen0, lab, rows, k):
    """Pack one core's rows. Empty region slots get x:=c (contributes 0);
    empty overflow slots get x:=centers[0], jo:=0 (contributes 0)."""
    m = lab[rows] - k * CPC
    order = np.argsort(m, kind="stable")
    rs = rows[order]
    ms = m[order]
    first = np.ones(len(ms), dtype=bool)
    first[1:] = ms[1:] != ms[:-1]

    xr = cs_k.copy()  # empty slots: x == c -> exactly 0
    xr[ms[first]] = x_bf[rs[first]]

    rest = rs[~first]
    ovf_rows = rest[:OVF]
    leftover = rest[OVF:]
    xo = np.broadcast_to(cen0, (OVF, D)).copy()
    jo = np.zeros(OVF, dtype=np.int32)
    xo[: len(ovf_rows)] = x_bf[ovf_rows]
    jo[: len(ovf_rows)] = (lab[ovf_rows] - k * CPC).astype(np.int32)
    return xr, xo, jo, leftover


def _run(x, labels, centers, **spmd_kwargs):
    import jax.numpy as jnp

    nc = _build()
    x = np.ascontiguousarray(np.asarray(x), dtype=np.float32)
    labels = np.ascontiguousarray(np.asarray(labels)).astype(np.int64)
    centers = np.ascontiguousarray(np.asarray(centers), dtype=np.float32)

    bf = jnp.bfloat16
    x_bf = np.asarray(jnp.asarray(x, dtype=bf))
    cen_bf = np.asarray(jnp.asarray(centers, dtype=bf))
    cs_pad = np.zeros((N_CORES, CPAD, D), dtype=cen_bf.dtype)
    cs_pad[:, :CPC] = cen_bf.reshape(N_CORES, CPC, D)

    owner = labels // CPC
    pending = [np.flatnonzero(owner == k) for k in range(N_CORES)]

    total = 0.0
    res = None
    while any(len(r) for r in pending):
        in_maps = []
        next_pending = []
        for k in range(N_CORES):
            xr, xo, jo, leftover = _pack_core(
                x_bf, cs_pad[k], cs_pad[k][0], labels, pending[k], k
            )
            cs3 = cs_pad[k].reshape(P, S, D)
            xr3 = xr.reshape(P, S, D)
            in_maps.append(
                {
                    "cxa": np.ascontiguousarray(
                        np.stack([cs3[:, 0:5], xr3[:, 0:5]], axis=2)
                    ),
                    "cxb": np.ascontiguousarray(
                        np.stack([cs3[:, 5:8], xr3[:, 5:8]], axis=2)
                    ),
                    "cxc": np.ascontiguousarray(
                        np.stack([cs3[:, 8:10], xr3[:, 8:10]], axis=2)
                    ),
                    "xovf": xo,
                    "jovf": jo,
                    "cshard": cs_pad[k],
                }
            )
            next_pending.append(leftover)
        res = run_bass_kernel_spmd(nc, in_maps, list(range(N_CORES)), **spmd_kwargs)
        for k in range(N_CORES):
            total += res.results[k]["sums"].astype(np.float64).sum()
        pending = next_pending
        spmd_kwargs = {}

    loss = (total + B * (C - 1) * _CLIP_LO) / B
    return np.asarray(loss, dtype=np.float32), res


def kernel(x, labels, centers):
    loss, _ = _run(x, labels, centers)
    return loss
